# revision 53
# baseline (speedup 1.0000x reference)
"""Multi-head attention forward on 8 Trainium2 NeuronCores.

Strategy: pure data-parallel over batch (B=8 -> 1 batch element per core,
no collectives). Per core, one fused kernel computes
    y = softmax((x Wq + bq)(x Wk + bk)^T / sqrt(hd)) (x Wv + bv) @ Wp + bp
for x [1024, 768], H=12 heads of 64 dims.

The ScalarE exp stream (96 x [128,1024] tiles, ~1.11us each, ~107us
total) is the pipeline heartbeat; all PE/DVE/DMA work hides under it:
  - Startup is DMA-queue-bound (per-queue ~25-65 GB/s): x rows split
    across all three DMA-capable queues (sync/scalar f32 + ScalarE
    casts, gpsimd direct cast-DMA); W_qkv in column slices (gpsimd
    cast-DMA via a rearranged AP) ordered pair-0 Q/K -> (x gate) ->
    pair 1 -> V -> pairs 2-5 -> W_proj, so pair-0 scores start ~25us in.
  - x^T via PE transposes, two per PSUM tile, one DVE drain per pair.
  - QKV bias applied as a K=1 matmul accumulation (b row x ones row),
    replacing per-column scatter DMAs and DVE tensor_scalar adds.
  - Per stage g: scores for pair g emitted first within each kb step
    (2x row-tiled, K=64 at base partitions 0/64); one filler chain
    threads through all stages carrying V-gen, next-pair QKV, prev-pair
    AV + normalization, and projection partials, paced n_fill per kb.
  - AV per head: lhsT = V_ext [128, 65] (ones column -> softmax sums Z
    in row 64), accumulated over kb in PSUM; AV of the current stage's
    own pair self-paces against exp-tile availability.
  - Normalization: per pair, Z rows staged to partition rows
    {0,32,64,96}, one batched Ln + Exp(-x) on ScalarE -> 1/Z (bf16),
    broadcast via K=1 matmuls (explicit tile_position), one DVE
    multiply per q half. No DRAM round trip.
  - proj: pairs 0-2 accumulate during stages 3-4, pairs 3-4 + bias
    during stage 5 (into y01, bf16); pair 5 + final adds + stores
    (3 DMA queues) form a short tail. Pair-5 normalization is split per
    q half so the first tail chains start before the last AV drains.
Compute dtype bf16 (fp32 PSUM accumulation).

kernel() reruns until two consecutive executions agree bitwise: the
first execution after NEFF load can start consumers before cold-path
DMAs land (observed flaky corruption); later runs are deterministic.
"""

import sys

for _p in ("/opt/trn_rl_repo", "/root/.axon_site/_ro/trn_rl_repo"):
    if _p not in sys.path:
        sys.path.append(_p)

import numpy as np

import concourse.bacc as bacc
import concourse.mybir as mybir
import concourse.tile as tile
from concourse.bass_utils import run_bass_kernel_spmd
from concourse.masks import make_identity

N_CORES = 8
P = 128
S = 1024
D = 768
H = 12
HD = 64
ND = D // P            # 6 d_model chunks
NS = S // P            # 8 seq tiles
NM = (2 * D) // P      # 12 M-tiles over Q,K douts
SCALE = 1.0 / (HD ** 0.5)
BF = mybir.dt.bfloat16
F32 = mybir.dt.float32
AF = mybir.ActivationFunctionType
ALU = mybir.AluOpType

_cached = None


def _patch_act_tables():
    """Force every Exp activation onto one table set so the table is
    loaded once."""
    import concourse.bacc as _bacc
    if getattr(_bacc, "_act_tables_patched", False):
        return
    orig = _bacc.get_activation_tables

    def patched(arch):
        tables = dict(orig(arch))
        for name, fns in tables.items():
            if name != "natural_log_exp_and_others":
                tables[name] = fns - {AF.Exp, AF.Ln}
        return tables

    _bacc.get_activation_tables = patched
    _bacc._act_tables_patched = True


def _build():
    _patch_act_tables()
    nc = bacc.Bacc("TRN2", target_bir_lowering=False, debug=False,
                   enable_asserts=True, num_devices=N_CORES)

    x_ext = nc.dram_tensor("x", [S, D], F32, kind="ExternalInput").ap()
    wq_ext = nc.dram_tensor("W_qkv", [D, 3 * D], F32, kind="ExternalInput").ap()
    bq_ext = nc.dram_tensor("b_qkv", [1, 3 * D], F32, kind="ExternalInput").ap()
    wp_ext = nc.dram_tensor("W_proj", [D, D], F32, kind="ExternalInput").ap()
    bp_ext = nc.dram_tensor("b_proj", [1, D], F32, kind="ExternalInput").ap()
    out_ext = nc.dram_tensor("out", [S, D], F32, kind="ExternalOutput").ap()

    with tile.TileContext(nc) as tc:
        _body(nc, tc, x_ext, wq_ext, bq_ext, wp_ext, bp_ext, out_ext)

    nc.compile()
    return nc


def _body(nc, tc, x_ext, wq_ext, bq_ext, wp_ext, bp_ext, out_ext):
    from contextlib import ExitStack
    from concourse.tile import add_dep_helper
    import itertools

    with ExitStack() as ctx:
        persist = ctx.enter_context(tc.tile_pool(name="persist", bufs=1))
        yout = ctx.enter_context(tc.tile_pool(name="yout", bufs=4))
        ps_mm = ctx.enter_context(tc.tile_pool(name="ps_mm", bufs=2, space="PSUM"))

        ident = persist.tile([P, P], BF)
        make_identity(nc, ident)
        xT = persist.tile([P, ND, S], BF)

        # warm the PE clock gate (HAM) during the DMA-bound startup:
        # ~4.5us of back-to-back dummy matmuls flips K=4/8 -> K=8/8 before
        # the transposes and pair-0 QKV arrive, so they run at 2.4 GHz
        with tc.tile_pool(name="warm", bufs=1, space="PSUM") as warm_pool:
            wps = warm_pool.tile([P, P], F32)
            for _ in range(35):
                nc.tensor.matmul(wps, ident, ident, start=True, stop=True)

        # ---- constants / biases (all row vectors; qkv bias is applied
        # with a K=1 matmul accumulation, no scatter DMAs needed) ----
        bq_row = persist.tile([1, 2 * D], BF)
        nc.gpsimd.dma_start(bq_row, bq_ext[0:1, 0:2 * D])
        bv_bf = persist.tile([1, D], BF)
        nc.gpsimd.dma_start(bv_bf, bq_ext[0:1, 2 * D:3 * D])
        bp_bf = persist.tile([1, D], BF)
        nc.gpsimd.dma_start(bp_bf, bp_ext[0:1, :])
        ones1 = persist.tile([1, 512], BF)
        nc.vector.memset(ones1, 1.0)
        ones4 = persist.tile([97, HD], BF)
        for r in (0, 32, 64, 96):
            nc.vector.memset(ones4[r:r + 1, :], 1.0)
        vext = persist.tile([P, NS, H * 65], BF)
        for sb in range(NS):
            vd = vext[:, sb, :].rearrange("p (h c) -> p h c", c=65)
            nc.vector.memset(vd[:, :, 64:65], 1.0)

        # ---- weights: gpsimd cast-DMA f32->bf16, column-sliced, ordered
        # so pair-0 Q/K land first, then V, then the rest ----
        w_bf = persist.tile([P, ND, 3 * D], BF)
        wp_bf = persist.tile([P, ND, D], BF)     # row chunk g = head pair g

        w_dma = {}

        def w_cols(c0, cn, key):
            src = wq_ext[0:D, c0:c0 + cn].rearrange("(kc p) c -> p kc c", p=P)
            h = nc.gpsimd.dma_start(w_bf[:, :, c0:c0 + cn], src)
            w_dma[key] = h
            return h

        w_chain = w_cols(0 * P, P, ("q", 0))             # Q pair 0
        nxt = w_cols(D + 0 * P, P, ("k", 0))             # K pair 0
        add_dep_helper(nxt.ins, w_chain.ins, reason="w slice order")
        w_chain = nxt

        # ---- x: f32 DMA split across sync+scalar queues; casts alternate
        # ScalarE/DVE; 6 transposes per seq block share one PSUM bank so a
        # single DVE copy drains them ----
        x_dma_last = None
        with tc.tile_pool(name="xin", bufs=3) as xin, \
             tc.tile_pool(name="ps_tr", bufs=6, space="PSUM") as ps_tr:
            for sb in range(NS):
                x_b = xin.tile([P, D], BF, tag="x_b")
                if sb % 3 == 2:
                    # gpsimd cast-DMA writes bf16 directly
                    nc.gpsimd.dma_start(x_b, x_ext[sb * P:(sb + 1) * P, :])
                else:
                    x_f = xin.tile([P, D], F32, tag="x_f")
                    eng = nc.sync if sb % 3 == 0 else nc.scalar
                    x_dma_last = eng.dma_start(
                        x_f, x_ext[sb * P:(sb + 1) * P, :])
                    nc.scalar.activation(x_b[:, 0:D // 2], x_f[:, 0:D // 2],
                                         AF.Copy)
                    nc.scalar.activation(x_b[:, D // 2:D], x_f[:, D // 2:D],
                                         AF.Copy)
                for kc in range(0, ND, 2):
                    pt = ps_tr.tile([P, 2, P], BF, tag="ps_tr")
                    for j in range(2):
                        nc.tensor.transpose(pt[:, j, :],
                                            x_b[:, (kc + j) * P:(kc + j + 1) * P],
                                            ident)
                    nc.vector.tensor_copy(xT[:, kc:kc + 2, sb * P:(sb + 1) * P],
                                          pt)

        # remaining weights stream after x is fully in (pair 1 first --
        # stage 0 needs it before V)
        first_rest = w_cols(1 * P, P, ("q", 1))
        add_dep_helper(first_rest.ins, w_chain.ins, reason="w slice order")
        add_dep_helper(first_rest.ins, x_dma_last.ins, reason="x before bulk W")
        w_chain = first_rest
        for c0, cn, key in [(D + 1 * P, P, ("k", 1)),
                            (2 * D, 512, ("v", 0)),      # V cols 0:512
                            (2 * D + 512, 256, ("v", 1)),  # V cols 512:768
                            (2 * P, P, ("q", 2)), (D + 2 * P, P, ("k", 2)),
                            (3 * P, P, ("q", 3)), (D + 3 * P, P, ("k", 3)),
                            (4 * P, P, ("q", 4)), (D + 4 * P, P, ("k", 4)),
                            (5 * P, P, ("q", 5)), (D + 5 * P, P, ("k", 5))]:
            nxt = w_cols(c0, cn, key)
            add_dep_helper(nxt.ins, w_chain.ins, reason="w slice order")
            w_chain = nxt
        for g in range(ND):
            wp_h = nc.gpsimd.dma_start(wp_bf[:, g, :],
                                       wp_ext[g * P:(g + 1) * P, :])
            w_dma[("wp", g)] = wp_h
            add_dep_helper(wp_h.ins, w_chain.ins,
                           reason="proj weights after qkv weights")

        expp = ctx.enter_context(tc.tile_pool(name="expp", bufs=30))
        sums_p = ctx.enter_context(tc.tile_pool(name="sums", bufs=1))
        ps_sc = ctx.enter_context(tc.tile_pool(name="ps_sc", bufs=2, space="PSUM"))
        ps_av = ctx.enter_context(tc.tile_pool(name="ps_av", bufs=2, space="PSUM"))

        qkT = persist.tile([P, NM, S], BF)
        aoT = persist.tile([P, ND, S], BF)   # paired attn out^T: pair g rows
        y01 = persist.tile([P, NS, D], BF)   # proj partial (pairs 0-4 + bias)

        # ---------------- generators (emission-paced fillers) ----------
        def gen_qkT(g):
            """Q^T/K^T tiles for pair g, one yield per PE instruction.
            First matmul of each group takes an explicit dep on its W
            column-slice DMA (the rearranged write AP may elude subtile
            dependency tracking)."""
            for m, nh in ((g, 0), (ND + g, 0), (g, 1), (ND + g, 1)):
                wkey = ("q", m) if m < ND else ("k", m - ND)
                ps = ps_mm.tile([P, 512], F32, tag="ps_mm",
                                name=f"qk{m}_{nh}")
                for kc in range(ND):
                    mm = nc.tensor.matmul(ps,
                                          w_bf[:, kc, m * P:(m + 1) * P],
                                          xT[:, kc, nh * 512:(nh + 1) * 512],
                                          start=(kc == 0), stop=False)
                    if kc == 0:
                        add_dep_helper(mm.ins, w_dma[wkey].ins,
                                       reason="W slice before qkT")
                    yield
                nc.tensor.matmul(ps, bq_row[0:1, m * P:(m + 1) * P],
                                 ones1[0:1, :], start=False, stop=True)
                yield
                nc.vector.tensor_copy(qkT[:, m, nh * 512:(nh + 1) * 512], ps)
                yield

        def gen_v(sb0, sb1):
            """V rows for seq blocks [sb0, sb1): s-major with the softmax
            ones column per head."""
            for sb in range(sb0, sb1):
                for c0, cn in ((0, 512), (512, 256)):
                    ps = ps_mm.tile([P, 512], F32, tag="ps_mm",
                                    name=f"v{sb}_{c0}")
                    for kc in range(ND):
                        mm = nc.tensor.matmul(
                            ps[:, :cn],
                            xT[:, kc, sb * P:(sb + 1) * P],
                            w_bf[:, kc, 2 * D + c0:2 * D + c0 + cn],
                            start=(kc == 0), stop=False)
                        if kc == 0:
                            add_dep_helper(mm.ins,
                                           w_dma[("v", c0 // 512)].ins,
                                           reason="W_v slice before V")
                        yield
                    nc.tensor.matmul(ps[:, :cn], ones1[0:1, 0:P], bv_bf[:, c0:c0 + cn],
                                     start=False, stop=True)
                    yield
                    h0 = c0 // HD
                    nh_h = cn // HD
                    vsrc = ps[:, :cn].rearrange("p (h c) -> p h c", c=HD)
                    vdst = vext[:, sb, :].rearrange("p (h c) -> p h c", c=65)
                    nc.vector.tensor_copy(vdst[:, h0:h0 + nh_h, 0:HD], vsrc)
                    yield

        ZROW = {(0, 0): 0, (0, 1): 32, (1, 0): 64, (1, 1): 96}

        def av_chain(g, half, qh, zb):
            """Full AV accumulation for (pair g, head half, q half) plus
            its epilogue: Z row staged into zb (partition row per combo),
            attn-out rows to aoT. Yields per PE instruction."""
            h = 2 * g + half
            qs = slice(qh * 512, (qh + 1) * 512)
            po = ps_av.tile([65, 512], F32, tag="ps_av", name=f"po{h}_{qh}")
            exps = stage_exps[g]
            for kb in range(NS):
                # self-pacing: if this pair's exps aren't emitted yet
                # (AV of the current stage's own pair), idle this filler
                # slot instead of overrunning the list
                while len(exps[half]) <= kb:
                    yield
                nc.tensor.matmul(po,
                                 vext[:, kb, h * 65:(h + 1) * 65],
                                 exps[half][kb][:, qs],
                                 start=(kb == 0), stop=(kb == NS - 1))
                yield
            rows = slice(half * HD, (half + 1) * HD)
            r = ZROW[(half, qh)]
            nc.vector.tensor_copy(zb[r:r + 1, :], po[64:65, :])
            nc.vector.tensor_copy(aoT[rows, g, qs], po[0:64, :])
            yield

        zb_of = {}

        def av_gen(g, combos=None):
            """AV accumulation for the given (half, qh) combos of pair g.
            The epilogues free pair-g exp tiles, so this runs early in
            each stage's filler chain."""
            if g not in zb_of:
                zb_of[g] = sums_p.tile([97, 512], F32, tag="zb", bufs=2,
                                       name=f"zb{g}")
            zb = zb_of[g]
            for half, qh in (combos or ALL_COMBOS):
                for _ in av_chain(g, half, qh, zb):
                    yield

        def norm_gen(g, qhs=(0, 1)):
            """One batched Ln+Exp(-x) on ScalarE produces 1/Z at partition
            rows {0,32,64,96}, broadcast with K=1 matmuls and applied with
            one DVE multiply per q half. Emitted late in the chain: its
            ps_mm (pb) tiles must follow any V/qkT tiles that AV work
            depends on, or the 2-buf ring's WAR deps would form a cycle."""
            zb = zb_of[g]
            lnz = sums_p.tile([97, 512], F32, tag="lnz", bufs=2,
                              name=f"lnz{g}")
            rec = sums_p.tile([97, 512], BF, tag="rec", bufs=2,
                              name=f"rec{g}")
            nc.scalar.activation(lnz, zb, AF.Ln)
            nc.scalar.activation(rec, lnz, AF.Exp, scale=-1.0)
            yield
            for qh in qhs:
                qs = slice(qh * 512, (qh + 1) * 512)
                pb = ps_mm.tile([P, 512], F32, tag="ps_mm",
                                name=f"pb{g}_{qh}")
                for half in range(2):
                    rows = slice(half * HD, (half + 1) * HD)
                    r = ZROW[(half, qh)]
                    nc.tensor.matmul(pb[rows, :], ones4[r:r + 1, :],
                                     rec[r:r + 1, :], start=True, stop=True,
                                     tile_position=(r, half * HD))
                    yield
                nc.vector.tensor_mul(aoT[:, g, qs], aoT[:, g, qs], pb)
                yield

        def gen_proj(sb_list, g0, g1, bias, accum):
            """proj partial sums over pairs [g0, g1) (+ b_proj if bias),
            landed in y01: copied on the first partial, accumulated on
            the second."""
            for sb in sb_list:
                pj0 = ps_mm.tile([P, 512], F32, tag="ps_mm", name=f"pj0_{sb}")
                pj1 = ps_mm.tile([P, 512], F32, tag="ps_mm", name=f"pj1_{sb}")
                for g in range(g0, g1):
                    last = (g == g1 - 1) and not bias
                    mm0 = nc.tensor.matmul(pj0,
                                           aoT[:, g, sb * P:(sb + 1) * P],
                                           wp_bf[:, g, 0:512],
                                           start=(g == g0), stop=last)
                    if sb == sb_list[0]:
                        add_dep_helper(mm0.ins, w_dma[("wp", g)].ins,
                                       reason="W_proj before proj")
                    yield
                    nc.tensor.matmul(pj1[:, 0:256],
                                     aoT[:, g, sb * P:(sb + 1) * P],
                                     wp_bf[:, g, 512:768],
                                     start=(g == g0), stop=last)
                    yield
                if bias:
                    nc.tensor.matmul(pj0, ones1[0:1, 0:P], bp_bf[:, 0:512],
                                     start=False, stop=True)
                    nc.tensor.matmul(pj1[:, 0:256], ones1[0:1, 0:P],
                                     bp_bf[:, 512:768],
                                     start=False, stop=True)
                    yield
                if accum:
                    nc.vector.tensor_tensor(y01[:, sb, 0:512], pj0,
                                            y01[:, sb, 0:512], op=ALU.add)
                    nc.vector.tensor_tensor(y01[:, sb, 512:768],
                                            pj1[:, 0:256],
                                            y01[:, sb, 512:768], op=ALU.add)
                else:
                    nc.vector.tensor_copy(y01[:, sb, 0:512], pj0)
                    nc.vector.tensor_copy(y01[:, sb, 512:768], pj1[:, 0:256])
                yield

        # ---------------- stages ----------------
        _END = object()
        stage_exps = {}
        rzbs = {}

        def stage(g, fillers, n_fill=12):
            """Scores+exp for pair g, emitted FIRST within each kb step so
            the exp stream never waits behind filler work; fillers are
            paced behind the scores and unconsumed items carry over to the
            next stage (returned)."""
            e0 = []
            e1 = []
            stage_exps[g] = (e0, e1)
            for kb in range(NS):
                ps0 = ps_sc.tile([P, S], F32, tag="ps_sc", name=f"sc0_{g}_{kb}")
                ps1 = ps_sc.tile([P, S], F32, tag="ps_sc", name=f"sc1_{g}_{kb}")
                for qh in range(2):
                    qs = slice(qh * 512, (qh + 1) * 512)
                    nc.tensor.matmul(ps0[:, qs],
                                     qkT[0:HD, ND + g, kb * P:(kb + 1) * P],
                                     qkT[0:HD, g, qs], start=True, stop=True)
                    nc.tensor.matmul(ps1[:, qs],
                                     qkT[HD:P, ND + g, kb * P:(kb + 1) * P],
                                     qkT[HD:P, g, qs], start=True, stop=True)
                t0 = expp.tile([P, S], BF, tag="expT", name=f"e0_{g}_{kb}")
                t1 = expp.tile([P, S], BF, tag="expT", name=f"e1_{g}_{kb}")
                nc.scalar.activation(t0, ps0, AF.Exp, scale=SCALE)
                nc.scalar.activation(t1, ps1, AF.Exp, scale=SCALE)
                e0.append(t0)
                e1.append(t1)
                for _ in range(n_fill):
                    if next(fillers, _END) is _END:
                        break
            return fillers

        ALL_COMBOS = ((0, 0), (0, 1), (1, 0), (1, 1))

        # emit pair-0 Q/K immediately (gated only on x + first W slices)
        for _ in gen_qkT(0):
            pass

        # One filler stream flows through all stages; unconsumed items
        # carry into the next stage's kb slots, so stage boundaries never
        # bunch PE work in front of the next stage's scores. Segment
        # order keeps the 2-buf ps_mm ring's WAR deps pointing forward
        # (V tiles before the norm pb tiles that AV feeds, etc.).
        def roundrobin(*gens):
            live = list(gens)
            while live:
                nxt = []
                for gen in live:
                    if next(gen, _END) is not _END:
                        nxt.append(gen)
                        yield
                live = nxt

        f = stage(0, itertools.chain(gen_qkT(1), gen_v(0, 5)), n_fill=12)
        f = stage(1, itertools.chain(f, gen_v(5, NS), gen_qkT(2),
                                     av_gen(0), norm_gen(0)), n_fill=12)
        f = stage(2, itertools.chain(f, gen_qkT(3), av_gen(1),
                                     norm_gen(1)), n_fill=12)
        # stages 3-4 also fold in the pair 0-2 projection partials
        f = stage(3, itertools.chain(f, gen_qkT(4), av_gen(2),
                                     norm_gen(2),
                                     gen_proj(range(0, 4), 0, 3,
                                              bias=False, accum=False)),
                  n_fill=13)
        f = stage(4, itertools.chain(f, gen_qkT(5), av_gen(3),
                                     norm_gen(3),
                                     gen_proj(range(4, NS), 0, 3,
                                              bias=False, accum=False)),
                  n_fill=13)
        # stage 5: AV(4)+norm(4), then pair 3-4 projection (+bias)
        # accumulating into y01, with AV(5) self-paced against its exps;
        # pair-5 norm is split per q-half so the qh0 tail chains can
        # start while the qh1 AV chains still drain
        g5 = ND - 1
        av5 = itertools.chain(av_gen(g5, ((0, 0), (1, 0))),
                              norm_gen(g5, (0,)),
                              av_gen(g5, ((0, 1), (1, 1))),
                              norm_gen(g5, (1,)))
        f = stage(g5,
                  itertools.chain(f, av_gen(ND - 2), norm_gen(ND - 2),
                                  roundrobin(av5,
                                             gen_proj(range(NS), 3, 5,
                                                      bias=True, accum=True))),
                  n_fill=16)
        for _ in f:
            pass

        # ---- tail: pair-5 proj contribution + final add + store.
        # ps_sc tiles are free after the last exp; one [P,768] PSUM tile
        # and a single DVE add per seq block keeps the tail short. ----
        for sb in range(NS):
            g = ND - 1
            pt = ps_sc.tile([P, S], F32, tag="ps_sc", name=f"pt_{sb}")
            mmt = nc.tensor.matmul(pt[:, 0:512], aoT[:, g, sb * P:(sb + 1) * P],
                                   wp_bf[:, g, 0:512], start=True, stop=True)
            if sb == 0:
                add_dep_helper(mmt.ins, w_dma[("wp", g)].ins,
                               reason="W_proj before tail proj")
            nc.tensor.matmul(pt[:, 512:768], aoT[:, g, sb * P:(sb + 1) * P],
                             wp_bf[:, g, 512:768], start=True, stop=True)
            y_sb = yout.tile([P, D], F32, tag="y")
            nc.vector.tensor_tensor(y_sb, pt[:, 0:D], y01[:, sb, :],
                                    op=ALU.add)
            eng = (nc.sync, nc.scalar, nc.gpsimd)[sb % 3]
            eng.dma_start(out_ext[sb * P:(sb + 1) * P, :], y_sb)


def kernel(**inputs):
    global _cached
    x = np.ascontiguousarray(np.asarray(inputs["x"], dtype=np.float32))
    w_qkv = np.ascontiguousarray(np.asarray(inputs["W_qkv"], dtype=np.float32))
    b_qkv = np.ascontiguousarray(np.asarray(inputs["b_qkv"], dtype=np.float32)).reshape(1, -1)
    w_proj = np.ascontiguousarray(np.asarray(inputs["W_proj"], dtype=np.float32))
    b_proj = np.ascontiguousarray(np.asarray(inputs["b_proj"], dtype=np.float32)).reshape(1, -1)

    if _cached is None:
        _cached = _build()
    nc = _cached

    in_maps = [{"x": x[b], "W_qkv": w_qkv, "b_qkv": b_qkv,
                "W_proj": w_proj, "b_proj": b_proj} for b in range(N_CORES)]
    last_err = None
    out = None
    prev = None
    for _attempt in range(5):
        try:
            res = run_bass_kernel_spmd(nc, in_maps,
                                       core_ids=list(range(N_CORES)))
            cur = np.stack([res.results[i]["out"] for i in range(N_CORES)],
                           axis=0)
            if np.isfinite(cur).all():
                # accept once two consecutive runs agree (guards against
                # cold-start DMA timing flakes on the very first execution)
                if prev is not None and np.array_equal(prev, cur):
                    return cur
                prev = cur
                out = cur
            else:
                prev = None
                last_err = ValueError("non-finite output, retrying")
        except Exception as e:  # transient NRT device errors happen rarely
            last_err = e
            prev = None
            import time
            time.sleep(2.0)
    if out is not None:
        return out
    raise last_err


# revision 55
# speedup vs baseline: 1.0018x; 1.0018x over previous
"""Multi-head attention forward on 8 Trainium2 NeuronCores.

Strategy: pure data-parallel over batch (B=8 -> 1 batch element per core,
no collectives). Per core, one fused kernel computes
    y = softmax((x Wq + bq)(x Wk + bk)^T / sqrt(hd)) (x Wv + bv) @ Wp + bp
for x [1024, 768], H=12 heads of 64 dims.

The ScalarE exp stream (96 x [128,1024] tiles, ~1.11us each, ~107us
total) is the pipeline heartbeat; all PE/DVE/DMA work hides under it:
  - Startup is DMA-queue-bound (per-queue ~25-65 GB/s): x rows split
    across all three DMA-capable queues (sync/scalar f32 + ScalarE
    casts, gpsimd direct cast-DMA); W_qkv in column slices (gpsimd
    cast-DMA via a rearranged AP) ordered pair-0 Q/K -> (x gate) ->
    pair 1 -> V -> pairs 2-5 -> W_proj, so pair-0 scores start ~25us in.
  - x^T via PE transposes, two per PSUM tile, one DVE drain per pair.
  - QKV bias applied as a K=1 matmul accumulation (b row x ones row),
    replacing per-column scatter DMAs and DVE tensor_scalar adds.
  - Per stage g: scores for pair g emitted first within each kb step
    (2x row-tiled, K=64 at base partitions 0/64); one filler chain
    threads through all stages carrying V-gen, next-pair QKV, prev-pair
    AV + normalization, and projection partials, paced n_fill per kb.
  - AV per head: lhsT = V_ext [128, 65] (ones column -> softmax sums Z
    in row 64), accumulated over kb in PSUM; AV of the current stage's
    own pair self-paces against exp-tile availability.
  - Normalization: per pair, Z rows staged to partition rows
    {0,32,64,96}, one batched Ln + Exp(-x) on ScalarE -> 1/Z (bf16),
    broadcast via K=1 matmuls (explicit tile_position), one DVE
    multiply per q half. No DRAM round trip.
  - proj: pairs 0-2 accumulate during stages 3-4, pairs 3-4 + bias
    during stage 5 (into y01, bf16); pair 5 + final adds + stores
    (3 DMA queues) form a short tail. Pair-5 normalization is split per
    q half so the first tail chains start before the last AV drains.
Compute dtype bf16 (fp32 PSUM accumulation).

kernel() reruns until two consecutive executions agree bitwise: the
first execution after NEFF load can start consumers before cold-path
DMAs land (observed flaky corruption); later runs are deterministic.
"""

import sys

for _p in ("/opt/trn_rl_repo", "/root/.axon_site/_ro/trn_rl_repo"):
    if _p not in sys.path:
        sys.path.append(_p)

import numpy as np

import concourse.bacc as bacc
import concourse.mybir as mybir
import concourse.tile as tile
from concourse.bass_utils import run_bass_kernel_spmd
from concourse.masks import make_identity

N_CORES = 8
P = 128
S = 1024
D = 768
H = 12
HD = 64
ND = D // P            # 6 d_model chunks
NS = S // P            # 8 seq tiles
NM = (2 * D) // P      # 12 M-tiles over Q,K douts
SCALE = 1.0 / (HD ** 0.5)
BF = mybir.dt.bfloat16
F32 = mybir.dt.float32
AF = mybir.ActivationFunctionType
ALU = mybir.AluOpType

_cached = None


def _patch_act_tables():
    """Force every Exp activation onto one table set so the table is
    loaded once."""
    import concourse.bacc as _bacc
    if getattr(_bacc, "_act_tables_patched", False):
        return
    orig = _bacc.get_activation_tables

    def patched(arch):
        tables = dict(orig(arch))
        for name, fns in tables.items():
            if name != "natural_log_exp_and_others":
                tables[name] = fns - {AF.Exp, AF.Ln}
        return tables

    _bacc.get_activation_tables = patched
    _bacc._act_tables_patched = True


def _build():
    _patch_act_tables()
    nc = bacc.Bacc("TRN2", target_bir_lowering=False, debug=False,
                   enable_asserts=True, num_devices=N_CORES)

    x_ext = nc.dram_tensor("x", [S, D], F32, kind="ExternalInput").ap()
    wq_ext = nc.dram_tensor("W_qkv", [D, 3 * D], F32, kind="ExternalInput").ap()
    bq_ext = nc.dram_tensor("b_qkv", [1, 3 * D], F32, kind="ExternalInput").ap()
    wp_ext = nc.dram_tensor("W_proj", [D, D], F32, kind="ExternalInput").ap()
    bp_ext = nc.dram_tensor("b_proj", [1, D], F32, kind="ExternalInput").ap()
    out_ext = nc.dram_tensor("out", [S, D], F32, kind="ExternalOutput").ap()

    with tile.TileContext(nc) as tc:
        _body(nc, tc, x_ext, wq_ext, bq_ext, wp_ext, bp_ext, out_ext)

    nc.compile()
    return nc


def _body(nc, tc, x_ext, wq_ext, bq_ext, wp_ext, bp_ext, out_ext):
    from contextlib import ExitStack
    from concourse.tile import add_dep_helper
    import itertools

    with ExitStack() as ctx:
        persist = ctx.enter_context(tc.tile_pool(name="persist", bufs=1))
        yout = ctx.enter_context(tc.tile_pool(name="yout", bufs=4))
        ps_mm = ctx.enter_context(tc.tile_pool(name="ps_mm", bufs=2, space="PSUM"))

        ident = persist.tile([P, P], BF)
        make_identity(nc, ident)
        xT = persist.tile([P, ND, S], BF)

        # ---- constants / biases (all row vectors; qkv bias is applied
        # with a K=1 matmul accumulation, no scatter DMAs needed) ----
        bq_row = persist.tile([1, 2 * D], BF)
        nc.gpsimd.dma_start(bq_row, bq_ext[0:1, 0:2 * D])
        bv_bf = persist.tile([1, D], BF)
        nc.gpsimd.dma_start(bv_bf, bq_ext[0:1, 2 * D:3 * D])
        bp_bf = persist.tile([1, D], BF)
        nc.gpsimd.dma_start(bp_bf, bp_ext[0:1, :])
        ones1 = persist.tile([1, 512], BF)
        nc.vector.memset(ones1, 1.0)
        ones4 = persist.tile([97, HD], BF)
        for r in (0, 32, 64, 96):
            nc.vector.memset(ones4[r:r + 1, :], 1.0)
        vext = persist.tile([P, NS, H * 65], BF)
        for sb in range(NS):
            vd = vext[:, sb, :].rearrange("p (h c) -> p h c", c=65)
            nc.vector.memset(vd[:, :, 64:65], 1.0)

        # ---- weights: gpsimd cast-DMA f32->bf16, column-sliced, ordered
        # so pair-0 Q/K land first, then V, then the rest ----
        w_bf = persist.tile([P, ND, 3 * D], BF)
        wp_bf = persist.tile([P, ND, D], BF)     # row chunk g = head pair g

        w_dma = {}

        def w_cols(c0, cn, key):
            src = wq_ext[0:D, c0:c0 + cn].rearrange("(kc p) c -> p kc c", p=P)
            h = nc.gpsimd.dma_start(w_bf[:, :, c0:c0 + cn], src)
            w_dma[key] = h
            return h

        w_chain = w_cols(0 * P, P, ("q", 0))             # Q pair 0
        nxt = w_cols(D + 0 * P, P, ("k", 0))             # K pair 0
        add_dep_helper(nxt.ins, w_chain.ins, reason="w slice order")
        w_chain = nxt

        # ---- x: f32 DMA split across sync+scalar queues; casts alternate
        # ScalarE/DVE; 6 transposes per seq block share one PSUM bank so a
        # single DVE copy drains them ----
        x_dma_last = None
        with tc.tile_pool(name="xin", bufs=3) as xin, \
             tc.tile_pool(name="ps_tr", bufs=6, space="PSUM") as ps_tr:
            for sb in range(NS):
                x_b = xin.tile([P, D], BF, tag="x_b")
                if sb % 3 == 2:
                    # gpsimd cast-DMA writes bf16 directly
                    nc.gpsimd.dma_start(x_b, x_ext[sb * P:(sb + 1) * P, :])
                else:
                    x_f = xin.tile([P, D], F32, tag="x_f")
                    eng = nc.sync if sb % 3 == 0 else nc.scalar
                    x_dma_last = eng.dma_start(
                        x_f, x_ext[sb * P:(sb + 1) * P, :])
                    nc.scalar.activation(x_b[:, 0:D // 2], x_f[:, 0:D // 2],
                                         AF.Copy)
                    nc.scalar.activation(x_b[:, D // 2:D], x_f[:, D // 2:D],
                                         AF.Copy)
                for kc in range(0, ND, 2):
                    pt = ps_tr.tile([P, 2, P], BF, tag="ps_tr")
                    for j in range(2):
                        nc.tensor.transpose(pt[:, j, :],
                                            x_b[:, (kc + j) * P:(kc + j + 1) * P],
                                            ident)
                    nc.vector.tensor_copy(xT[:, kc:kc + 2, sb * P:(sb + 1) * P],
                                          pt)

        # remaining weights stream after x is fully in (pair 1 first --
        # stage 0 needs it before V)
        first_rest = w_cols(1 * P, P, ("q", 1))
        add_dep_helper(first_rest.ins, w_chain.ins, reason="w slice order")
        add_dep_helper(first_rest.ins, x_dma_last.ins, reason="x before bulk W")
        w_chain = first_rest
        for c0, cn, key in [(D + 1 * P, P, ("k", 1)),
                            (2 * D, 512, ("v", 0)),      # V cols 0:512
                            (2 * D + 512, 256, ("v", 1)),  # V cols 512:768
                            (2 * P, P, ("q", 2)), (D + 2 * P, P, ("k", 2)),
                            (3 * P, P, ("q", 3)), (D + 3 * P, P, ("k", 3)),
                            (4 * P, P, ("q", 4)), (D + 4 * P, P, ("k", 4)),
                            (5 * P, P, ("q", 5)), (D + 5 * P, P, ("k", 5))]:
            nxt = w_cols(c0, cn, key)
            add_dep_helper(nxt.ins, w_chain.ins, reason="w slice order")
            w_chain = nxt
        for g in range(ND):
            wp_h = nc.gpsimd.dma_start(wp_bf[:, g, :],
                                       wp_ext[g * P:(g + 1) * P, :])
            w_dma[("wp", g)] = wp_h
            add_dep_helper(wp_h.ins, w_chain.ins,
                           reason="proj weights after qkv weights")

        expp = ctx.enter_context(tc.tile_pool(name="expp", bufs=30))
        sums_p = ctx.enter_context(tc.tile_pool(name="sums", bufs=1))
        ps_sc = ctx.enter_context(tc.tile_pool(name="ps_sc", bufs=2, space="PSUM"))
        ps_av = ctx.enter_context(tc.tile_pool(name="ps_av", bufs=2, space="PSUM"))

        qkT = persist.tile([P, NM, S], BF)
        aoT = persist.tile([P, ND, S], BF)   # paired attn out^T: pair g rows
        y01 = persist.tile([P, NS, D], BF)   # proj partial (pairs 0-4 + bias)

        # ---------------- generators (emission-paced fillers) ----------
        def gen_qkT(g):
            """Q^T/K^T tiles for pair g, one yield per PE instruction.
            First matmul of each group takes an explicit dep on its W
            column-slice DMA (the rearranged write AP may elude subtile
            dependency tracking)."""
            for m, nh in ((g, 0), (ND + g, 0), (g, 1), (ND + g, 1)):
                wkey = ("q", m) if m < ND else ("k", m - ND)
                ps = ps_mm.tile([P, 512], F32, tag="ps_mm",
                                name=f"qk{m}_{nh}")
                for kc in range(ND):
                    mm = nc.tensor.matmul(ps,
                                          w_bf[:, kc, m * P:(m + 1) * P],
                                          xT[:, kc, nh * 512:(nh + 1) * 512],
                                          start=(kc == 0), stop=False)
                    if kc == 0:
                        add_dep_helper(mm.ins, w_dma[wkey].ins,
                                       reason="W slice before qkT")
                    yield
                nc.tensor.matmul(ps, bq_row[0:1, m * P:(m + 1) * P],
                                 ones1[0:1, :], start=False, stop=True)
                yield
                nc.vector.tensor_copy(qkT[:, m, nh * 512:(nh + 1) * 512], ps)
                yield

        def gen_v(sb0, sb1):
            """V rows for seq blocks [sb0, sb1): s-major with the softmax
            ones column per head."""
            for sb in range(sb0, sb1):
                for c0, cn in ((0, 512), (512, 256)):
                    ps = ps_mm.tile([P, 512], F32, tag="ps_mm",
                                    name=f"v{sb}_{c0}")
                    for kc in range(ND):
                        mm = nc.tensor.matmul(
                            ps[:, :cn],
                            xT[:, kc, sb * P:(sb + 1) * P],
                            w_bf[:, kc, 2 * D + c0:2 * D + c0 + cn],
                            start=(kc == 0), stop=False)
                        if kc == 0:
                            add_dep_helper(mm.ins,
                                           w_dma[("v", c0 // 512)].ins,
                                           reason="W_v slice before V")
                        yield
                    nc.tensor.matmul(ps[:, :cn], ones1[0:1, 0:P], bv_bf[:, c0:c0 + cn],
                                     start=False, stop=True)
                    yield
                    h0 = c0 // HD
                    nh_h = cn // HD
                    vsrc = ps[:, :cn].rearrange("p (h c) -> p h c", c=HD)
                    vdst = vext[:, sb, :].rearrange("p (h c) -> p h c", c=65)
                    nc.vector.tensor_copy(vdst[:, h0:h0 + nh_h, 0:HD], vsrc)
                    yield

        ZROW = {(0, 0): 0, (0, 1): 32, (1, 0): 64, (1, 1): 96}

        def av_chain(g, half, qh, zb):
            """Full AV accumulation for (pair g, head half, q half) plus
            its epilogue: Z row staged into zb (partition row per combo),
            attn-out rows to aoT. Yields per PE instruction."""
            h = 2 * g + half
            qs = slice(qh * 512, (qh + 1) * 512)
            po = ps_av.tile([65, 512], F32, tag="ps_av", name=f"po{h}_{qh}")
            exps = stage_exps[g]
            for kb in range(NS):
                # self-pacing: if this pair's exps aren't emitted yet
                # (AV of the current stage's own pair), idle this filler
                # slot instead of overrunning the list
                while len(exps[half]) <= kb:
                    yield
                nc.tensor.matmul(po,
                                 vext[:, kb, h * 65:(h + 1) * 65],
                                 exps[half][kb][:, qs],
                                 start=(kb == 0), stop=(kb == NS - 1))
                yield
            rows = slice(half * HD, (half + 1) * HD)
            r = ZROW[(half, qh)]
            nc.vector.tensor_copy(zb[r:r + 1, :], po[64:65, :])
            nc.vector.tensor_copy(aoT[rows, g, qs], po[0:64, :])
            yield

        zb_of = {}

        def av_gen(g, combos=None):
            """AV accumulation for the given (half, qh) combos of pair g.
            The epilogues free pair-g exp tiles, so this runs early in
            each stage's filler chain."""
            if g not in zb_of:
                zb_of[g] = sums_p.tile([97, 512], F32, tag="zb", bufs=2,
                                       name=f"zb{g}")
            zb = zb_of[g]
            for half, qh in (combos or ALL_COMBOS):
                for _ in av_chain(g, half, qh, zb):
                    yield

        def norm_gen(g, qhs=(0, 1)):
            """One batched Ln+Exp(-x) on ScalarE produces 1/Z at partition
            rows {0,32,64,96}, broadcast with K=1 matmuls and applied with
            one DVE multiply per q half. Emitted late in the chain: its
            ps_mm (pb) tiles must follow any V/qkT tiles that AV work
            depends on, or the 2-buf ring's WAR deps would form a cycle."""
            zb = zb_of[g]
            lnz = sums_p.tile([97, 512], F32, tag="lnz", bufs=2,
                              name=f"lnz{g}")
            rec = sums_p.tile([97, 512], BF, tag="rec", bufs=2,
                              name=f"rec{g}")
            nc.scalar.activation(lnz, zb, AF.Ln)
            nc.scalar.activation(rec, lnz, AF.Exp, scale=-1.0)
            yield
            for qh in qhs:
                qs = slice(qh * 512, (qh + 1) * 512)
                pb = ps_mm.tile([P, 512], F32, tag="ps_mm",
                                name=f"pb{g}_{qh}")
                for half in range(2):
                    rows = slice(half * HD, (half + 1) * HD)
                    r = ZROW[(half, qh)]
                    nc.tensor.matmul(pb[rows, :], ones4[r:r + 1, :],
                                     rec[r:r + 1, :], start=True, stop=True,
                                     tile_position=(r, half * HD))
                    yield
                nc.vector.tensor_mul(aoT[:, g, qs], aoT[:, g, qs], pb)
                yield

        def gen_proj(sb_list, g0, g1, bias, accum):
            """proj partial sums over pairs [g0, g1) (+ b_proj if bias),
            landed in y01: copied on the first partial, accumulated on
            the second."""
            for sb in sb_list:
                pj0 = ps_mm.tile([P, 512], F32, tag="ps_mm", name=f"pj0_{sb}")
                pj1 = ps_mm.tile([P, 512], F32, tag="ps_mm", name=f"pj1_{sb}")
                for g in range(g0, g1):
                    last = (g == g1 - 1) and not bias
                    mm0 = nc.tensor.matmul(pj0,
                                           aoT[:, g, sb * P:(sb + 1) * P],
                                           wp_bf[:, g, 0:512],
                                           start=(g == g0), stop=last)
                    if sb == sb_list[0]:
                        add_dep_helper(mm0.ins, w_dma[("wp", g)].ins,
                                       reason="W_proj before proj")
                    yield
                    nc.tensor.matmul(pj1[:, 0:256],
                                     aoT[:, g, sb * P:(sb + 1) * P],
                                     wp_bf[:, g, 512:768],
                                     start=(g == g0), stop=last)
                    yield
                if bias:
                    nc.tensor.matmul(pj0, ones1[0:1, 0:P], bp_bf[:, 0:512],
                                     start=False, stop=True)
                    nc.tensor.matmul(pj1[:, 0:256], ones1[0:1, 0:P],
                                     bp_bf[:, 512:768],
                                     start=False, stop=True)
                    yield
                if accum:
                    nc.vector.tensor_tensor(y01[:, sb, 0:512], pj0,
                                            y01[:, sb, 0:512], op=ALU.add)
                    nc.vector.tensor_tensor(y01[:, sb, 512:768],
                                            pj1[:, 0:256],
                                            y01[:, sb, 512:768], op=ALU.add)
                else:
                    nc.vector.tensor_copy(y01[:, sb, 0:512], pj0)
                    nc.vector.tensor_copy(y01[:, sb, 512:768], pj1[:, 0:256])
                yield

        # ---------------- stages ----------------
        _END = object()
        stage_exps = {}
        rzbs = {}

        def stage(g, fillers, n_fill=12):
            """Scores+exp for pair g, emitted FIRST within each kb step so
            the exp stream never waits behind filler work; fillers are
            paced behind the scores and unconsumed items carry over to the
            next stage (returned)."""
            e0 = []
            e1 = []
            stage_exps[g] = (e0, e1)
            for kb in range(NS):
                ps0 = ps_sc.tile([P, S], F32, tag="ps_sc", name=f"sc0_{g}_{kb}")
                ps1 = ps_sc.tile([P, S], F32, tag="ps_sc", name=f"sc1_{g}_{kb}")
                for qh in range(2):
                    qs = slice(qh * 512, (qh + 1) * 512)
                    nc.tensor.matmul(ps0[:, qs],
                                     qkT[0:HD, ND + g, kb * P:(kb + 1) * P],
                                     qkT[0:HD, g, qs], start=True, stop=True)
                    nc.tensor.matmul(ps1[:, qs],
                                     qkT[HD:P, ND + g, kb * P:(kb + 1) * P],
                                     qkT[HD:P, g, qs], start=True, stop=True)
                t0 = expp.tile([P, S], BF, tag="expT", name=f"e0_{g}_{kb}")
                t1 = expp.tile([P, S], BF, tag="expT", name=f"e1_{g}_{kb}")
                nc.scalar.activation(t0, ps0, AF.Exp, scale=SCALE)
                nc.scalar.activation(t1, ps1, AF.Exp, scale=SCALE)
                e0.append(t0)
                e1.append(t1)
                for _ in range(n_fill):
                    if next(fillers, _END) is _END:
                        break
            return fillers

        ALL_COMBOS = ((0, 0), (0, 1), (1, 0), (1, 1))

        # emit pair-0 Q/K immediately (gated only on x + first W slices)
        for _ in gen_qkT(0):
            pass

        # One filler stream flows through all stages; unconsumed items
        # carry into the next stage's kb slots, so stage boundaries never
        # bunch PE work in front of the next stage's scores. Segment
        # order keeps the 2-buf ps_mm ring's WAR deps pointing forward
        # (V tiles before the norm pb tiles that AV feeds, etc.).
        def roundrobin(*gens):
            live = list(gens)
            while live:
                nxt = []
                for gen in live:
                    if next(gen, _END) is not _END:
                        nxt.append(gen)
                        yield
                live = nxt

        f = stage(0, itertools.chain(gen_qkT(1), gen_v(0, 5)), n_fill=12)
        f = stage(1, itertools.chain(f, gen_v(5, NS), gen_qkT(2),
                                     av_gen(0), norm_gen(0)), n_fill=11)
        f = stage(2, itertools.chain(f, gen_qkT(3), av_gen(1),
                                     norm_gen(1)), n_fill=11)
        # stages 3-4 also fold in the pair 0-2 projection partials
        f = stage(3, itertools.chain(f, gen_qkT(4), av_gen(2),
                                     norm_gen(2),
                                     gen_proj(range(0, 4), 0, 3,
                                              bias=False, accum=False)),
                  n_fill=12)
        f = stage(4, itertools.chain(f, gen_qkT(5), av_gen(3),
                                     norm_gen(3),
                                     gen_proj(range(4, NS), 0, 3,
                                              bias=False, accum=False)),
                  n_fill=12)
        # stage 5: AV(4)+norm(4), then pair 3-4 projection (+bias)
        # accumulating into y01, with AV(5) self-paced against its exps;
        # pair-5 norm is split per q-half so the qh0 tail chains can
        # start while the qh1 AV chains still drain
        g5 = ND - 1
        av5 = itertools.chain(av_gen(g5, ((0, 0), (1, 0))),
                              norm_gen(g5, (0,)),
                              av_gen(g5, ((0, 1), (1, 1))),
                              norm_gen(g5, (1,)))
        f = stage(g5,
                  itertools.chain(f, av_gen(ND - 2), norm_gen(ND - 2),
                                  roundrobin(av5,
                                             gen_proj(range(NS), 3, 5,
                                                      bias=True, accum=True))),
                  n_fill=16)
        for _ in f:
            pass

        # ---- tail: pair-5 proj contribution + final add + store.
        # ps_sc tiles are free after the last exp; one [P,768] PSUM tile
        # and a single DVE add per seq block keeps the tail short. ----
        for sb in range(NS):
            g = ND - 1
            pt = ps_sc.tile([P, S], F32, tag="ps_sc", name=f"pt_{sb}")
            mmt = nc.tensor.matmul(pt[:, 0:512], aoT[:, g, sb * P:(sb + 1) * P],
                                   wp_bf[:, g, 0:512], start=True, stop=True)
            if sb == 0:
                add_dep_helper(mmt.ins, w_dma[("wp", g)].ins,
                               reason="W_proj before tail proj")
            nc.tensor.matmul(pt[:, 512:768], aoT[:, g, sb * P:(sb + 1) * P],
                             wp_bf[:, g, 512:768], start=True, stop=True)
            y_sb = yout.tile([P, D], F32, tag="y")
            nc.vector.tensor_tensor(y_sb, pt[:, 0:D], y01[:, sb, :],
                                    op=ALU.add)
            eng = (nc.sync, nc.scalar, nc.gpsimd)[sb % 3]
            eng.dma_start(out_ext[sb * P:(sb + 1) * P, :], y_sb)


def kernel(**inputs):
    global _cached
    x = np.ascontiguousarray(np.asarray(inputs["x"], dtype=np.float32))
    w_qkv = np.ascontiguousarray(np.asarray(inputs["W_qkv"], dtype=np.float32))
    b_qkv = np.ascontiguousarray(np.asarray(inputs["b_qkv"], dtype=np.float32)).reshape(1, -1)
    w_proj = np.ascontiguousarray(np.asarray(inputs["W_proj"], dtype=np.float32))
    b_proj = np.ascontiguousarray(np.asarray(inputs["b_proj"], dtype=np.float32)).reshape(1, -1)

    if _cached is None:
        _cached = _build()
    nc = _cached

    in_maps = [{"x": x[b], "W_qkv": w_qkv, "b_qkv": b_qkv,
                "W_proj": w_proj, "b_proj": b_proj} for b in range(N_CORES)]
    last_err = None
    out = None
    prev = None
    for _attempt in range(5):
        try:
            res = run_bass_kernel_spmd(nc, in_maps,
                                       core_ids=list(range(N_CORES)))
            cur = np.stack([res.results[i]["out"] for i in range(N_CORES)],
                           axis=0)
            if np.isfinite(cur).all():
                # accept once two consecutive runs agree (guards against
                # cold-start DMA timing flakes on the very first execution)
                if prev is not None and np.array_equal(prev, cur):
                    return cur
                prev = cur
                out = cur
            else:
                prev = None
                last_err = ValueError("non-finite output, retrying")
        except Exception as e:  # transient NRT device errors happen rarely
            last_err = e
            prev = None
            import time
            time.sleep(2.0)
    if out is not None:
        return out
    raise last_err


# revision 56
# speedup vs baseline: 1.0118x; 1.0100x over previous
"""Multi-head attention forward on 8 Trainium2 NeuronCores.

Strategy: pure data-parallel over batch (B=8 -> 1 batch element per core,
no collectives). Per core, one fused kernel computes
    y = softmax((x Wq + bq)(x Wk + bk)^T / sqrt(hd)) (x Wv + bv) @ Wp + bp
for x [1024, 768], H=12 heads of 64 dims.

The ScalarE exp stream (96 x [128,1024] tiles, ~1.11us each, ~107us
total) is the pipeline heartbeat; all PE/DVE/DMA work hides under it:
  - Startup is DMA-queue-bound (per-queue ~25-65 GB/s): x rows split
    across all three DMA-capable queues (sync/scalar f32 + ScalarE
    casts, gpsimd direct cast-DMA); W_qkv in column slices (gpsimd
    cast-DMA via a rearranged AP) ordered pair-0 Q/K -> (x gate) ->
    pair 1 -> V -> pairs 2-5 -> W_proj, so pair-0 scores start ~25us in.
  - x^T via PE transposes, two per PSUM tile, one DVE drain per pair.
  - QKV bias applied as a K=1 matmul accumulation (b row x ones row),
    replacing per-column scatter DMAs and DVE tensor_scalar adds.
  - Per stage g: scores for pair g emitted first within each kb step
    (2x row-tiled, K=64 at base partitions 0/64); one filler chain
    threads through all stages carrying V-gen, next-pair QKV, prev-pair
    AV + normalization, and projection partials, paced n_fill per kb.
  - AV per head: lhsT = V_ext [128, 65] (ones column -> softmax sums Z
    in row 64), accumulated over kb in PSUM; AV of the current stage's
    own pair self-paces against exp-tile availability.
  - Normalization: per pair, Z rows staged to partition rows
    {0,32,64,96}, one batched Ln + Exp(-x) on ScalarE -> 1/Z (bf16),
    broadcast via K=1 matmuls (explicit tile_position), one DVE
    multiply per q half. No DRAM round trip.
  - proj: pairs 0-2 accumulate during stages 3-4, pairs 3-4 + bias
    during stage 5 (into y01, bf16); pair 5 + final adds + stores
    (3 DMA queues) form a short tail. Pair-5 normalization is split per
    q half so the first tail chains start before the last AV drains.
Compute dtype bf16 (fp32 PSUM accumulation).

kernel() reruns until two consecutive executions agree bitwise: the
first execution after NEFF load can start consumers before cold-path
DMAs land (observed flaky corruption); later runs are deterministic.
"""

import sys

for _p in ("/opt/trn_rl_repo", "/root/.axon_site/_ro/trn_rl_repo"):
    if _p not in sys.path:
        sys.path.append(_p)

import numpy as np

import concourse.bacc as bacc
import concourse.mybir as mybir
import concourse.tile as tile
from concourse.bass_utils import run_bass_kernel_spmd
from concourse.masks import make_identity

N_CORES = 8
P = 128
S = 1024
D = 768
H = 12
HD = 64
ND = D // P            # 6 d_model chunks
NS = S // P            # 8 seq tiles
NM = (2 * D) // P      # 12 M-tiles over Q,K douts
SCALE = 1.0 / (HD ** 0.5)
BF = mybir.dt.bfloat16
F32 = mybir.dt.float32
AF = mybir.ActivationFunctionType
ALU = mybir.AluOpType

_cached = None


def _patch_act_tables():
    """Force every Exp activation onto one table set so the table is
    loaded once."""
    import concourse.bacc as _bacc
    if getattr(_bacc, "_act_tables_patched", False):
        return
    orig = _bacc.get_activation_tables

    def patched(arch):
        tables = dict(orig(arch))
        for name, fns in tables.items():
            if name != "natural_log_exp_and_others":
                tables[name] = fns - {AF.Exp, AF.Ln}
        return tables

    _bacc.get_activation_tables = patched
    _bacc._act_tables_patched = True


def _build():
    _patch_act_tables()
    nc = bacc.Bacc("TRN2", target_bir_lowering=False, debug=False,
                   enable_asserts=True, num_devices=N_CORES)

    x_ext = nc.dram_tensor("x", [S, D], F32, kind="ExternalInput").ap()
    wq_ext = nc.dram_tensor("W_qkv", [D, 3 * D], F32, kind="ExternalInput").ap()
    bq_ext = nc.dram_tensor("b_qkv", [1, 3 * D], F32, kind="ExternalInput").ap()
    wp_ext = nc.dram_tensor("W_proj", [D, D], F32, kind="ExternalInput").ap()
    bp_ext = nc.dram_tensor("b_proj", [1, D], F32, kind="ExternalInput").ap()
    out_ext = nc.dram_tensor("out", [S, D], F32, kind="ExternalOutput").ap()

    with tile.TileContext(nc) as tc:
        _body(nc, tc, x_ext, wq_ext, bq_ext, wp_ext, bp_ext, out_ext)

    nc.compile()
    return nc


def _body(nc, tc, x_ext, wq_ext, bq_ext, wp_ext, bp_ext, out_ext):
    from contextlib import ExitStack
    from concourse.tile import add_dep_helper
    import itertools

    with ExitStack() as ctx:
        persist = ctx.enter_context(tc.tile_pool(name="persist", bufs=1))
        yout = ctx.enter_context(tc.tile_pool(name="yout", bufs=4))
        ps_mm = ctx.enter_context(tc.tile_pool(name="ps_mm", bufs=2, space="PSUM"))

        ident = persist.tile([P, P], BF)
        make_identity(nc, ident)
        xT = persist.tile([P, ND, S], BF)

        # ---- constants / biases (all row vectors; qkv bias is applied
        # with a K=1 matmul accumulation, no scatter DMAs needed) ----
        bq_row = persist.tile([1, 2 * D], BF)
        nc.gpsimd.dma_start(bq_row, bq_ext[0:1, 0:2 * D])
        bv_bf = persist.tile([1, D], BF)
        nc.gpsimd.dma_start(bv_bf, bq_ext[0:1, 2 * D:3 * D])
        bp_bf = persist.tile([1, D], BF)
        nc.gpsimd.dma_start(bp_bf, bp_ext[0:1, :])
        ones1 = persist.tile([1, 512], BF)
        nc.vector.memset(ones1, 1.0)
        ones4 = persist.tile([97, HD], BF)
        for r in (0, 32, 64, 96):
            nc.vector.memset(ones4[r:r + 1, :], 1.0)
        vext = persist.tile([P, NS, H * 65], BF)
        for sb in range(NS):
            vd = vext[:, sb, :].rearrange("p (h c) -> p h c", c=65)
            nc.vector.memset(vd[:, :, 64:65], 1.0)

        # ---- weights: gpsimd cast-DMA f32->bf16, column-sliced, ordered
        # so pair-0 Q/K land first, then V, then the rest ----
        w_bf = persist.tile([P, ND, 3 * D], BF)
        wp_bf = persist.tile([P, ND, D], BF)     # row chunk g = head pair g

        w_dma = {}

        def w_cols(c0, cn, key):
            src = wq_ext[0:D, c0:c0 + cn].rearrange("(kc p) c -> p kc c", p=P)
            h = nc.gpsimd.dma_start(w_bf[:, :, c0:c0 + cn], src)
            w_dma[key] = h
            return h

        w_chain = w_cols(0 * P, P, ("q", 0))             # Q pair 0
        nxt = w_cols(D + 0 * P, P, ("k", 0))             # K pair 0
        add_dep_helper(nxt.ins, w_chain.ins, reason="w slice order")
        w_chain = nxt

        # ---- x: f32 DMA split across sync+scalar queues; casts alternate
        # ScalarE/DVE; 6 transposes per seq block share one PSUM bank so a
        # single DVE copy drains them ----
        x_dma_last = None
        with tc.tile_pool(name="xin", bufs=3) as xin, \
             tc.tile_pool(name="ps_tr", bufs=6, space="PSUM") as ps_tr:
            for sb in range(NS):
                x_b = xin.tile([P, D], BF, tag="x_b")
                if sb % 3 == 2:
                    # gpsimd cast-DMA writes bf16 directly
                    nc.gpsimd.dma_start(x_b, x_ext[sb * P:(sb + 1) * P, :])
                else:
                    x_f = xin.tile([P, D], F32, tag="x_f")
                    eng = nc.sync if sb % 3 == 0 else nc.scalar
                    x_dma_last = eng.dma_start(
                        x_f, x_ext[sb * P:(sb + 1) * P, :])
                    nc.scalar.activation(x_b[:, 0:D // 2], x_f[:, 0:D // 2],
                                         AF.Copy)
                    nc.scalar.activation(x_b[:, D // 2:D], x_f[:, D // 2:D],
                                         AF.Copy)
                for kc in range(0, ND, 2):
                    pt = ps_tr.tile([P, 2, P], BF, tag="ps_tr")
                    for j in range(2):
                        nc.tensor.transpose(pt[:, j, :],
                                            x_b[:, (kc + j) * P:(kc + j + 1) * P],
                                            ident)
                    nc.vector.tensor_copy(xT[:, kc:kc + 2, sb * P:(sb + 1) * P],
                                          pt)

        # remaining weights stream after x is fully in (pair 1 first --
        # stage 0 needs it before V)
        first_rest = w_cols(1 * P, P, ("q", 1))
        add_dep_helper(first_rest.ins, w_chain.ins, reason="w slice order")
        add_dep_helper(first_rest.ins, x_dma_last.ins, reason="x before bulk W")
        w_chain = first_rest
        for c0, cn, key in [(D + 1 * P, P, ("k", 1)),
                            (2 * D, 512, ("v", 0)),      # V cols 0:512
                            (2 * D + 512, 256, ("v", 1)),  # V cols 512:768
                            (2 * P, P, ("q", 2)), (D + 2 * P, P, ("k", 2)),
                            (3 * P, P, ("q", 3)), (D + 3 * P, P, ("k", 3)),
                            (4 * P, P, ("q", 4)), (D + 4 * P, P, ("k", 4)),
                            (5 * P, P, ("q", 5)), (D + 5 * P, P, ("k", 5))]:
            nxt = w_cols(c0, cn, key)
            add_dep_helper(nxt.ins, w_chain.ins, reason="w slice order")
            w_chain = nxt
        for g in range(ND):
            wp_h = nc.gpsimd.dma_start(wp_bf[:, g, :],
                                       wp_ext[g * P:(g + 1) * P, :])
            w_dma[("wp", g)] = wp_h
            add_dep_helper(wp_h.ins, w_chain.ins,
                           reason="proj weights after qkv weights")

        expp = ctx.enter_context(tc.tile_pool(name="expp", bufs=30))
        sums_p = ctx.enter_context(tc.tile_pool(name="sums", bufs=1))
        ps_sc = ctx.enter_context(tc.tile_pool(name="ps_sc", bufs=2, space="PSUM"))
        ps_av = ctx.enter_context(tc.tile_pool(name="ps_av", bufs=2, space="PSUM"))

        qkT = persist.tile([P, NM, S], BF)
        aoT = persist.tile([P, ND, S], BF)   # paired attn out^T: pair g rows
        y01 = persist.tile([P, NS, D], BF)   # proj partial (pairs 0-4 + bias)

        # ---------------- generators (emission-paced fillers) ----------
        def gen_qkT(g):
            """Q^T/K^T tiles for pair g, one yield per PE instruction.
            First matmul of each group takes an explicit dep on its W
            column-slice DMA (the rearranged write AP may elude subtile
            dependency tracking)."""
            for m, nh in ((g, 0), (ND + g, 0), (g, 1), (ND + g, 1)):
                wkey = ("q", m) if m < ND else ("k", m - ND)
                ps = ps_mm.tile([P, 512], F32, tag="ps_mm",
                                name=f"qk{m}_{nh}")
                for kc in range(ND):
                    mm = nc.tensor.matmul(ps,
                                          w_bf[:, kc, m * P:(m + 1) * P],
                                          xT[:, kc, nh * 512:(nh + 1) * 512],
                                          start=(kc == 0), stop=False)
                    if kc == 0:
                        add_dep_helper(mm.ins, w_dma[wkey].ins,
                                       reason="W slice before qkT")
                    yield
                nc.tensor.matmul(ps, bq_row[0:1, m * P:(m + 1) * P],
                                 ones1[0:1, :], start=False, stop=True)
                yield
                nc.vector.tensor_copy(qkT[:, m, nh * 512:(nh + 1) * 512], ps)
                yield

        def gen_v(sb0, sb1):
            """V rows for seq blocks [sb0, sb1): s-major with the softmax
            ones column per head."""
            for sb in range(sb0, sb1):
                for c0, cn in ((0, 512), (512, 256)):
                    ps = ps_mm.tile([P, 512], F32, tag="ps_mm",
                                    name=f"v{sb}_{c0}")
                    for kc in range(ND):
                        mm = nc.tensor.matmul(
                            ps[:, :cn],
                            xT[:, kc, sb * P:(sb + 1) * P],
                            w_bf[:, kc, 2 * D + c0:2 * D + c0 + cn],
                            start=(kc == 0), stop=False)
                        if kc == 0:
                            add_dep_helper(mm.ins,
                                           w_dma[("v", c0 // 512)].ins,
                                           reason="W_v slice before V")
                        yield
                    nc.tensor.matmul(ps[:, :cn], ones1[0:1, 0:P], bv_bf[:, c0:c0 + cn],
                                     start=False, stop=True)
                    yield
                    h0 = c0 // HD
                    nh_h = cn // HD
                    vsrc = ps[:, :cn].rearrange("p (h c) -> p h c", c=HD)
                    vdst = vext[:, sb, :].rearrange("p (h c) -> p h c", c=65)
                    nc.vector.tensor_copy(vdst[:, h0:h0 + nh_h, 0:HD], vsrc)
                    yield

        ZROW = {(0, 0): 0, (0, 1): 32, (1, 0): 64, (1, 1): 96}

        def av_chain(g, half, qh, zb):
            """Full AV accumulation for (pair g, head half, q half) plus
            its epilogue: Z row staged into zb (partition row per combo),
            attn-out rows to aoT. Yields per PE instruction."""
            h = 2 * g + half
            qs = slice(qh * 512, (qh + 1) * 512)
            po = ps_av.tile([65, 512], F32, tag="ps_av", name=f"po{h}_{qh}")
            exps = stage_exps[g]
            for kb in range(NS):
                # self-pacing: if this pair's exps aren't emitted yet
                # (AV of the current stage's own pair), idle this filler
                # slot instead of overrunning the list
                while len(exps[half]) <= kb:
                    yield
                nc.tensor.matmul(po,
                                 vext[:, kb, h * 65:(h + 1) * 65],
                                 exps[half][kb][:, qs],
                                 start=(kb == 0), stop=(kb == NS - 1))
                yield
            rows = slice(half * HD, (half + 1) * HD)
            r = ZROW[(half, qh)]
            nc.vector.tensor_copy(zb[r:r + 1, :], po[64:65, :])
            nc.vector.tensor_copy(aoT[rows, g, qs], po[0:64, :])
            yield

        zb_of = {}

        def av_gen(g, combos=None):
            """AV accumulation for the given (half, qh) combos of pair g.
            The epilogues free pair-g exp tiles, so this runs early in
            each stage's filler chain."""
            if g not in zb_of:
                zb_of[g] = sums_p.tile([97, 512], F32, tag="zb", bufs=2,
                                       name=f"zb{g}")
            zb = zb_of[g]
            for half, qh in (combos or ALL_COMBOS):
                for _ in av_chain(g, half, qh, zb):
                    yield

        def norm_gen(g, qhs=(0, 1)):
            """One batched Ln+Exp(-x) on ScalarE produces 1/Z at partition
            rows {0,32,64,96}, broadcast with K=1 matmuls and applied with
            one DVE multiply per q half. Emitted late in the chain: its
            ps_mm (pb) tiles must follow any V/qkT tiles that AV work
            depends on, or the 2-buf ring's WAR deps would form a cycle."""
            zb = zb_of[g]
            lnz = sums_p.tile([97, 512], F32, tag="lnz", bufs=2,
                              name=f"lnz{g}")
            rec = sums_p.tile([97, 512], BF, tag="rec", bufs=2,
                              name=f"rec{g}")
            nc.scalar.activation(lnz, zb, AF.Ln)
            nc.scalar.activation(rec, lnz, AF.Exp, scale=-1.0)
            yield
            for qh in qhs:
                qs = slice(qh * 512, (qh + 1) * 512)
                pb = ps_mm.tile([P, 512], F32, tag="ps_mm",
                                name=f"pb{g}_{qh}")
                for half in range(2):
                    rows = slice(half * HD, (half + 1) * HD)
                    r = ZROW[(half, qh)]
                    nc.tensor.matmul(pb[rows, :], ones4[r:r + 1, :],
                                     rec[r:r + 1, :], start=True, stop=True,
                                     tile_position=(r, half * HD))
                    yield
                nc.vector.tensor_mul(aoT[:, g, qs], aoT[:, g, qs], pb)
                yield

        def gen_proj(sb_list, g0, g1, bias, accum):
            """proj partial sums over pairs [g0, g1) (+ b_proj if bias),
            landed in y01: copied on the first partial, accumulated on
            the second."""
            for sb in sb_list:
                pj0 = ps_mm.tile([P, 512], F32, tag="ps_mm", name=f"pj0_{sb}")
                pj1 = ps_mm.tile([P, 512], F32, tag="ps_mm", name=f"pj1_{sb}")
                for g in range(g0, g1):
                    last = (g == g1 - 1) and not bias
                    mm0 = nc.tensor.matmul(pj0,
                                           aoT[:, g, sb * P:(sb + 1) * P],
                                           wp_bf[:, g, 0:512],
                                           start=(g == g0), stop=last)
                    if sb == sb_list[0]:
                        add_dep_helper(mm0.ins, w_dma[("wp", g)].ins,
                                       reason="W_proj before proj")
                    yield
                    nc.tensor.matmul(pj1[:, 0:256],
                                     aoT[:, g, sb * P:(sb + 1) * P],
                                     wp_bf[:, g, 512:768],
                                     start=(g == g0), stop=last)
                    yield
                if bias:
                    nc.tensor.matmul(pj0, ones1[0:1, 0:P], bp_bf[:, 0:512],
                                     start=False, stop=True)
                    nc.tensor.matmul(pj1[:, 0:256], ones1[0:1, 0:P],
                                     bp_bf[:, 512:768],
                                     start=False, stop=True)
                    yield
                if accum:
                    nc.vector.tensor_tensor(y01[:, sb, 0:512], pj0,
                                            y01[:, sb, 0:512], op=ALU.add)
                    nc.vector.tensor_tensor(y01[:, sb, 512:768],
                                            pj1[:, 0:256],
                                            y01[:, sb, 512:768], op=ALU.add)
                else:
                    nc.vector.tensor_copy(y01[:, sb, 0:512], pj0)
                    nc.vector.tensor_copy(y01[:, sb, 512:768], pj1[:, 0:256])
                yield

        # ---------------- stages ----------------
        _END = object()
        stage_exps = {}
        rzbs = {}

        def stage(g, fillers, n_fill=12):
            """Scores+exp for pair g, emitted FIRST within each kb step so
            the exp stream never waits behind filler work; fillers are
            paced behind the scores and unconsumed items carry over to the
            next stage (returned)."""
            e0 = []
            e1 = []
            stage_exps[g] = (e0, e1)
            for kb in range(NS):
                ps0 = ps_sc.tile([P, S], F32, tag="ps_sc", name=f"sc0_{g}_{kb}")
                ps1 = ps_sc.tile([P, S], F32, tag="ps_sc", name=f"sc1_{g}_{kb}")
                for qh in range(2):
                    qs = slice(qh * 512, (qh + 1) * 512)
                    nc.tensor.matmul(ps0[:, qs],
                                     qkT[0:HD, ND + g, kb * P:(kb + 1) * P],
                                     qkT[0:HD, g, qs], start=True, stop=True)
                    nc.tensor.matmul(ps1[:, qs],
                                     qkT[HD:P, ND + g, kb * P:(kb + 1) * P],
                                     qkT[HD:P, g, qs], start=True, stop=True)
                t0 = expp.tile([P, S], BF, tag="expT", name=f"e0_{g}_{kb}")
                t1 = expp.tile([P, S], BF, tag="expT", name=f"e1_{g}_{kb}")
                nc.scalar.activation(t0, ps0, AF.Exp, scale=SCALE)
                nc.scalar.activation(t1, ps1, AF.Exp, scale=SCALE)
                e0.append(t0)
                e1.append(t1)
                for _ in range(n_fill):
                    if next(fillers, _END) is _END:
                        break
            return fillers

        ALL_COMBOS = ((0, 0), (0, 1), (1, 0), (1, 1))

        # emit pair-0 Q/K immediately (gated only on x + first W slices)
        for _ in gen_qkT(0):
            pass

        # One filler stream flows through all stages; unconsumed items
        # carry into the next stage's kb slots, so stage boundaries never
        # bunch PE work in front of the next stage's scores. Segment
        # order keeps the 2-buf ps_mm ring's WAR deps pointing forward
        # (V tiles before the norm pb tiles that AV feeds, etc.).
        def roundrobin(*gens):
            live = list(gens)
            while live:
                nxt = []
                for gen in live:
                    if next(gen, _END) is not _END:
                        nxt.append(gen)
                        yield
                live = nxt

        f = stage(0, itertools.chain(gen_qkT(1), gen_v(0, 5)), n_fill=12)
        f = stage(1, itertools.chain(f, gen_v(5, NS), gen_qkT(2),
                                     av_gen(0), norm_gen(0)), n_fill=12)
        f = stage(2, itertools.chain(f, gen_qkT(3), av_gen(1),
                                     norm_gen(1)), n_fill=12)
        # stages 3-4 also fold in the pair 0-2 projection partials
        f = stage(3, itertools.chain(f, gen_qkT(4), av_gen(2),
                                     norm_gen(2),
                                     gen_proj(range(0, 4), 0, 3,
                                              bias=False, accum=False)),
                  n_fill=13)
        f = stage(4, itertools.chain(f, gen_qkT(5), av_gen(3),
                                     norm_gen(3),
                                     gen_proj(range(4, NS), 0, 3,
                                              bias=False, accum=False)),
                  n_fill=13)
        # stage 5: AV(4)+norm(4), then pair 3-4 projection (+bias)
        # accumulating into y01, with AV(5) self-paced against its exps;
        # pair-5 norm is split per q-half so the qh0 tail chains can
        # start while the qh1 AV chains still drain
        g5 = ND - 1
        av5 = itertools.chain(av_gen(g5, ((0, 0), (1, 0))),
                              norm_gen(g5, (0,)),
                              av_gen(g5, ((0, 1), (1, 1))),
                              norm_gen(g5, (1,)))
        f = stage(g5,
                  itertools.chain(f, av_gen(ND - 2), norm_gen(ND - 2),
                                  roundrobin(av5,
                                             gen_proj(range(NS), 3, 5,
                                                      bias=True, accum=True))),
                  n_fill=16)
        for _ in f:
            pass

        # ---- tail: pair-5 proj contribution + final add + store.
        # ps_sc tiles are free after the last exp; one [P,768] PSUM tile
        # and a single DVE add per seq block keeps the tail short. ----
        for sb in range(NS):
            g = ND - 1
            pt = ps_sc.tile([P, S], F32, tag="ps_sc", name=f"pt_{sb}")
            mmt = nc.tensor.matmul(pt[:, 0:512], aoT[:, g, sb * P:(sb + 1) * P],
                                   wp_bf[:, g, 0:512], start=True, stop=True)
            if sb == 0:
                add_dep_helper(mmt.ins, w_dma[("wp", g)].ins,
                               reason="W_proj before tail proj")
            nc.tensor.matmul(pt[:, 512:768], aoT[:, g, sb * P:(sb + 1) * P],
                             wp_bf[:, g, 512:768], start=True, stop=True)
            y_sb = yout.tile([P, D], F32, tag="y")
            nc.vector.tensor_tensor(y_sb, pt[:, 0:D], y01[:, sb, :],
                                    op=ALU.add)
            eng = (nc.sync, nc.scalar, nc.gpsimd)[sb % 3]
            eng.dma_start(out_ext[sb * P:(sb + 1) * P, :], y_sb)


def kernel(**inputs):
    global _cached
    x = np.ascontiguousarray(np.asarray(inputs["x"], dtype=np.float32))
    w_qkv = np.ascontiguousarray(np.asarray(inputs["W_qkv"], dtype=np.float32))
    b_qkv = np.ascontiguousarray(np.asarray(inputs["b_qkv"], dtype=np.float32)).reshape(1, -1)
    w_proj = np.ascontiguousarray(np.asarray(inputs["W_proj"], dtype=np.float32))
    b_proj = np.ascontiguousarray(np.asarray(inputs["b_proj"], dtype=np.float32)).reshape(1, -1)

    if _cached is None:
        _cached = _build()
    nc = _cached

    in_maps = [{"x": x[b], "W_qkv": w_qkv, "b_qkv": b_qkv,
                "W_proj": w_proj, "b_proj": b_proj} for b in range(N_CORES)]
    last_err = None
    out = None
    prev = None
    for _attempt in range(5):
        try:
            res = run_bass_kernel_spmd(nc, in_maps,
                                       core_ids=list(range(N_CORES)))
            cur = np.stack([res.results[i]["out"] for i in range(N_CORES)],
                           axis=0)
            if np.isfinite(cur).all():
                # accept once two consecutive runs agree (guards against
                # cold-start DMA timing flakes on the very first execution)
                if prev is not None and np.array_equal(prev, cur):
                    return cur
                prev = cur
                out = cur
            else:
                prev = None
                last_err = ValueError("non-finite output, retrying")
        except Exception as e:  # transient NRT device errors happen rarely
            last_err = e
            prev = None
            import time
            time.sleep(2.0)
    if out is not None:
        return out
    raise last_err
